# revision 1
# baseline (speedup 1.0000x reference)
"""Trainium2 Bass kernel for MeshLaplacianLoss.

Computes  sum((L @ verts)**2) / B  for L [9216,9216] f32, verts [8,9216,3] f32.

Strategy: row-shard the output over 8 cores. Core m computes rows
[m*1152, (m+1)*1152) of lv = L @ V, where V is verts flattened to
[9216, 24] (batch*xyz as columns).  Using out^T = V^T @ L[:, cols]
(valid because the mesh Laplacian is symmetric; verified on host with a
transpose fallback), the big operand L streams through the PE as the
moving operand with no transposes anywhere:

    lhsT (stationary) = V K-tile  [128, M]
    rhs  (moving)     = L K-tile  [128, 384] x 3 chunks
    out  (PSUM)       = lv^T chunk [M, 384] accumulated over 72 K-tiles

Default mode "bf16x2": L is cast to bf16 on the host (mesh Laplacian
entries are small integers -> exact in bf16; verified, with fp32
fallback), and V is split into bf16 hi + bf16 lo whose partial products
accumulate side by side in fp32 PSUM (M = 48 = 24 hi | 24 lo columns).
lv = hi-part + lo-part restores ~fp32 accuracy while halving HBM bytes
for the dominant L stream.  "fp32" mode is bit-serious exact (4
cycles/row on the PE) and is also the fallback for non-bf16-exact L.

Epilogue squares and free-dim-reduces to [24, 3] per core; host sums
partials in float64 and divides by B.

Written in raw Bass (explicit semaphores, hand-rolled multi-buffering):
the Tile scheduler's generated sync puts >1 semaphore wait on single
instructions, which this container's walrus rejects.
"""

import sys

for _p in ("/opt/trn_rl_repo",):
    if _p not in sys.path:
        sys.path.insert(0, _p)

import numpy as np

N = 9216
B = 8
NCORES = 8
SHARD = N // NCORES          # 1152 output rows per core
P = 128                      # partitions
KTILES = N // P              # 72
M = B * 3                    # 24 output columns of lv^T
NCHUNK = 3
CHUNK = SHARD // NCHUNK      # 384

# parts: how many scaled components V is split into (cols = parts*24);
# GROUP K-tiles ride in one dma_start ([128, GROUP*1152]) to stay past the
# ~1 MiB DMA efficiency knee; BUFS slots give the prefetch depth.
# Component k of V is stored as dtype(v_k * SPLIT_SCALE**k) and the
# accumulators are recombined as sum_k out_k / SPLIT_SCALE**k.
SPLIT_SCALE = 16.0
_MODES = {
    "fp32": dict(parts=1, group=2, bufs=6),
    "bf16x2": dict(parts=2, group=4, bufs=6),
    "fp8x4": dict(parts=4, group=8, bufs=6),
}

_cache = {}


def _build_nc(dtype_mode, loops=1):
    import concourse.bass as bass
    import concourse.mybir as mybir

    cfg = _MODES[dtype_mode]
    PARTS, GROUP, BUFS = cfg["parts"], cfg["group"], cfg["bufs"]
    MSTAT = PARTS * M
    NGROUPS = KTILES // GROUP
    dt_data = {
        "fp32": mybir.dt.float32,
        "bf16x2": mybir.dt.bfloat16,
        "fp8x4": mybir.dt.float8e4,
    }[dtype_mode]
    f32 = mybir.dt.float32
    GS = GROUP * SHARD
    split = PARTS > 1

    nc = bass.Bass()
    lcols = nc.declare_dram_parameter("lcols", [NGROUPS, P, GS], dt_data, isOutput=False)
    vstat = nc.declare_dram_parameter("vstat", [P, KTILES * MSTAT], dt_data, isOutput=False)
    out = nc.declare_dram_parameter("partial", [M, NCHUNK], f32, isOutput=True)

    with (
        nc.sbuf_tensor([P, KTILES * MSTAT], dt_data) as v_sb,
        nc.sbuf_tensor([P, BUFS * GS], dt_data) as l_sb,
        nc.psum_tensor([MSTAT, CHUNK], f32) as acc0,
        nc.psum_tensor([MSTAT, CHUNK], f32) as acc1,
        nc.psum_tensor([MSTAT, CHUNK], f32) as acc2,
        nc.sbuf_tensor([MSTAT, NCHUNK * CHUNK], f32) as cp_sb,
        nc.sbuf_tensor([M, max(PARTS - 1, 1) * NCHUNK * CHUNK], f32) as lo_sb,
        nc.sbuf_tensor([M, CHUNK], f32) as sq_sb,
        nc.sbuf_tensor([M, NCHUNK], f32) as red_sb,
        nc.semaphore("dma_sem") as dma_sem,
        nc.semaphore("pe_sem") as pe_sem,
        nc.semaphore("dvec_sem") as dvec_sem,
        nc.semaphore("dve_sem") as dve_sem,
        nc.semaphore("out_sem") as out_sem,
    ):
        accs = [acc0, acc1, acc2]
        NG = NGROUPS * loops
        MM_PER_G = GROUP * NCHUNK

        with nc.Block() as block:

            @block.sync
            def _(sync):
                sync.dma_start(v_sb[:], vstat[:]).then_inc(dma_sem, 16)
                for gu in range(NG):
                    u = gu % NGROUPS
                    if gu >= BUFS:
                        sync.wait_ge(pe_sem, MM_PER_G * (gu - BUFS + 1))
                    slot = gu % BUFS
                    sync.dma_start(
                        l_sb[:, slot * GS : (slot + 1) * GS], lcols[u]
                    ).then_inc(dma_sem, 16)
                if split:
                    # shift the lo accumulators down to partitions 0..23
                    sync.wait_ge(dvec_sem, NCHUNK)
                    for k in range(1, PARTS):
                        for j in range(NCHUNK):
                            o = ((k - 1) * NCHUNK + j) * CHUNK
                            sync.dma_start(
                                lo_sb[:, o : o + CHUNK],
                                cp_sb[k * M : (k + 1) * M, j * CHUNK : (j + 1) * CHUNK],
                            ).then_inc(dma_sem, 16)
                sync.wait_ge(dve_sem, 1)
                # Reset all waited-on semaphores BEFORE the out DMA: the
                # runtime can report execution done at out-buffer readiness,
                # so anything after the out DMA races the next execution of
                # the same loaded NEFF (sems are never cleared by the
                # runtime).  The out DMA gets its own never-waited sem.
                nshift = (PARTS - 1) * NCHUNK if split else 0
                sync.wait_ge(dma_sem, 16 * (1 + NG + nshift))
                for s in (dma_sem, pe_sem, dvec_sem, dve_sem):
                    sync.sem_clear(s)
                sync.dma_start(out[:], red_sb[:]).then_inc(out_sem, 16)

            @block.tensor
            def _(tensor):
                for gu in range(NG):
                    u = gu % NGROUPS
                    slot = gu % BUFS
                    tensor.wait_ge(dma_sem, 16 * (gu + 2))
                    for t_in in range(GROUP):
                        t = u * GROUP + t_in
                        for j in range(NCHUNK):
                            tensor.matmul(
                                accs[j][:],
                                v_sb[:, t * MSTAT : (t + 1) * MSTAT],
                                l_sb[
                                    :,
                                    slot * GS
                                    + t_in * SHARD
                                    + j * CHUNK : slot * GS
                                    + t_in * SHARD
                                    + (j + 1) * CHUNK,
                                ],
                                start=(t == 0),
                                stop=(t == KTILES - 1),
                            ).then_inc(pe_sem, 1)

            @block.vector
            def _(vector):
                vector.wait_ge(pe_sem, MM_PER_G * NG)
                if split:
                    for j in range(NCHUNK):
                        vector.tensor_copy(
                            cp_sb[:, j * CHUNK : (j + 1) * CHUNK], accs[j][:]
                        ).then_inc(dvec_sem, 1)
                    # lo parts arrive via the SP shift DMAs
                    nshift = (PARTS - 1) * NCHUNK
                    vector.wait_ge(dma_sem, 16 * (NG + 1 + nshift))
                    for j in range(NCHUNK):
                        acc = cp_sb[0:M, j * CHUNK : (j + 1) * CHUNK]
                        sc = SPLIT_SCALE if dtype_mode == "fp8x4" else 1.0
                        for k in range(1, PARTS):
                            o = ((k - 1) * NCHUNK + j) * CHUNK
                            lo = lo_sb[:, o : o + CHUNK]
                            if sc != 1.0:
                                vector.tensor_scalar_mul(lo, lo, 1.0 / sc**k)
                            vector.tensor_add(lo, acc, lo)
                            acc = lo
                        vector.tensor_mul(sq_sb[:], acc, acc)
                        red = vector.reduce_sum(
                            red_sb[:, j : j + 1], sq_sb[:], axis=mybir.AxisListType.X
                        )
                        if j == NCHUNK - 1:
                            red.then_inc(dve_sem, 1)
                else:
                    for j in range(NCHUNK):
                        cp = cp_sb[:, j * CHUNK : (j + 1) * CHUNK]
                        vector.tensor_copy(cp, accs[j][:])
                        vector.tensor_mul(sq_sb[:], cp, cp)
                        red = vector.reduce_sum(
                            red_sb[:, j : j + 1], sq_sb[:], axis=mybir.AxisListType.X
                        )
                        if j == NCHUNK - 1:
                            red.then_inc(dve_sem, 1)

    return nc


def _get_nc(dtype_mode, loops=1):
    key = (dtype_mode, loops)
    if key not in _cache:
        _cache[key] = _build_nc(dtype_mode, loops)
    return _cache[key]


def _symmetric_sample(L, n=200000, seed=0):
    rng = np.random.default_rng(seed)
    i = rng.integers(0, L.shape[0], n)
    j = rng.integers(0, L.shape[1], n)
    return bool(np.array_equal(L[i, j], L[j, i]))


def _prepare_inputs(laplacian, verts, dtype_mode):
    import ml_dtypes

    cfg = _MODES[dtype_mode]
    GROUP = cfg["group"]
    NGROUPS = KTILES // GROUP
    GS = GROUP * SHARD

    L = np.asarray(laplacian, dtype=np.float32)
    V = np.asarray(verts, dtype=np.float32)
    assert L.shape == (N, N) and V.shape == (B, N, 3)

    # rhs tiles need L^T columns; mesh Laplacians are symmetric so we can
    # slice L directly.  Sampled check with a transposed fallback keeps the
    # kernel correct for arbitrary (non-symmetric) inputs.
    Lsrc = L if _symmetric_sample(L) else np.ascontiguousarray(L.T)

    V24 = V.transpose(1, 0, 2).reshape(N, M)                    # [9216, 24]
    if dtype_mode == "fp32":
        vstat = np.ascontiguousarray(
            V24.reshape(KTILES, P, M).transpose(1, 0, 2)
        ).reshape(P, -1)
        Lcast = Lsrc
    else:
        dt = ml_dtypes.bfloat16 if dtype_mode == "bf16x2" else ml_dtypes.float8_e4m3
        sc = SPLIT_SCALE if dtype_mode == "fp8x4" else 1.0
        parts = _MODES[dtype_mode]["parts"]
        comps, resid = [], V24.copy()
        for k in range(parts):
            c = (resid * sc**k).astype(dt)
            comps.append(c.reshape(KTILES, P, M))
            resid = resid - c.astype(np.float32) / sc**k
        stat = np.concatenate(comps, axis=2)                     # [72,128,parts*24]
        vstat = np.ascontiguousarray(stat.transpose(1, 0, 2)).reshape(P, -1)
        Lcast = Lsrc.astype(dt)

    in_maps = []
    for c in range(NCORES):
        lc = np.ascontiguousarray(Lcast[:, c * SHARD : (c + 1) * SHARD])
        # interleave GROUP K-tiles side by side in the free dim
        lc = lc.reshape(NGROUPS, GROUP, P, SHARD).transpose(0, 2, 1, 3)
        lc = np.ascontiguousarray(lc).reshape(NGROUPS, P, GS)
        in_maps.append({"lcols": lc, "vstat": vstat})
    return in_maps


def _exact_in(L, dt):
    return bool(np.array_equal(L.astype(dt).astype(np.float32), L))


def kernel(laplacian, verts, _dtype_mode=None, _loops=1):
    import ml_dtypes
    from concourse.bass_utils import run_bass_kernel_spmd

    L = np.asarray(laplacian, dtype=np.float32)
    if _dtype_mode is None:
        # The reduced-dtype kernels are ~fp32-accurate only when L's entries
        # are exactly representable (true for mesh Laplacians: small
        # integers).  Otherwise fall back to the exact fp32 kernel.
        if _exact_in(L, ml_dtypes.float8_e4m3):
            _dtype_mode = "fp8x4"
        elif _exact_in(L, ml_dtypes.bfloat16):
            _dtype_mode = "bf16x2"
        else:
            _dtype_mode = "fp32"

    in_maps = _prepare_inputs(L, verts, _dtype_mode)
    nc = _get_nc(_dtype_mode, _loops)
    res = run_bass_kernel_spmd(nc, in_maps, core_ids=list(range(NCORES)))
    total = np.float64(0.0)
    for r in res.results:
        total += r["partial"].astype(np.float64).sum()
    return np.float32(total / B)



# revision 13
# speedup vs baseline: 5.6928x; 5.6928x over previous
"""Trainium2 Bass kernel for MeshLaplacianLoss.

Computes  sum((L @ verts)**2) / B  for L [9216,9216] f32, verts [8,9216,3] f32.

Fast path "tri" (used when L is detected block-tridiagonal at 128
granularity and exactly representable in fp8e4m3 — true for mesh
Laplacians, whose 7 diagonals all sit within +-96 of the main): row-shard
the 9216 output rows over 8 cores (9 blocks of 128 rows each).  For
output block j only the three 128-wide column blocks k = j-1, j, j+1 of
L are nonzero, so core c streams just 27 transposed fp8 128x128 blocks
(432 KB vs 10.6 MB for the dense row-shard) and runs 27 accumulating
matmuls:

    lhsT (stationary) = L[j-rows, k-cols]^T  [128, 128] fp8
    rhs  (moving)     = V[k-block]           [128, 48]  fp8 (hi | lo*16)
    out  (PSUM)       = lv[j-block]          [128, 2, 9, 24] f32, one bank

V is split into fp8 hi + fp8 lo-scaled components (columns side by side)
to restore ~1e-5 relative accuracy; the whole per-core output lives in a
single PSUM bank, accumulated with one start / one stop (PSUM pending-
zero is bank granular).  Epilogue: combine hi + lo/16, square, free-dim
reduce to [128, 1] per core; host sums partials in float64 / B.

Dense fallback (any other L): the original full-matmul kernel, mode
fp8x4 / bf16x2 / fp32 chosen by exactness of L in the narrow dtype.

Written in raw Bass (explicit semaphores, hand-rolled multi-buffering):
the Tile scheduler's generated sync puts >1 semaphore wait on single
instructions, which this container's walrus rejects.
"""

import sys

for _p in ("/opt/trn_rl_repo",):
    if _p not in sys.path:
        sys.path.insert(0, _p)

import numpy as np

N = 9216
B = 8
NCORES = 8
SHARD = N // NCORES          # 1152 output rows per core
P = 128                      # partitions
KTILES = N // P              # 72
M = B * 3                    # 24 output columns of lv^T
NCHUNK = 3
CHUNK = SHARD // NCHUNK      # 384

# ---- tri (block-tridiagonal) path constants ----
JB = SHARD // P              # 9 output blocks per core
KB = JB + 2                  # 11 k-blocks incl halo
VPARTS = 2                   # fp8 hi + fp8 lo*16
MV = VPARTS * M              # 48 moving columns
TRI_BUFS = 2                 # L-block multibuffer depth (in groups)
TRI_GROUP = 8                # reps per dma_start / sem round-trip
TRI_SCALE = 16.0
# off-diagonal blocks only have bands within +-96 of the main diagonal:
# the kap=0 (sub) L^T blocks are zero in partitions < 32, the kap=2
# (super) blocks in partitions >= 96, so they ship as [96, 128] tiles.
TRI_SUB_BASE = 32            # kap=0 tiles live in partitions 32..128
TRI_SUP_SIZE = 96            # kap=2 tiles live in partitions 0..96

# parts: how many scaled components V is split into (cols = parts*24);
# GROUP K-tiles ride in one dma_start ([128, GROUP*1152]) to stay past the
# ~1 MiB DMA efficiency knee; BUFS slots give the prefetch depth.
# Component k of V is stored as dtype(v_k * SPLIT_SCALE**k) and the
# accumulators are recombined as sum_k out_k / SPLIT_SCALE**k.
SPLIT_SCALE = 16.0
_MODES = {
    "fp32": dict(parts=1, group=2, bufs=6),
    "bf16x2": dict(parts=2, group=4, bufs=6),
    "fp8x4": dict(parts=4, group=8, bufs=6),
}

_cache = {}


# --------------------------------------------------------------------------
# tri path: block-tridiagonal L, 27 fp8 128x128 blocks per core
# --------------------------------------------------------------------------

def _build_nc_tri(loops=1, group=TRI_GROUP, bufs=TRI_BUFS):
    import concourse.bass as bass
    import concourse.mybir as mybir

    fp8 = mybir.dt.float8e4
    f32 = mybir.dt.float32
    SUB0 = TRI_SUB_BASE      # 32
    SUBP = P - SUB0          # 96 partitions used by kap=0 tiles
    SUPP = TRI_SUP_SIZE      # 96 partitions used by kap=2 tiles
    LW1 = JB * P             # 1152 cols of diag tiles per rep
    # rep sizes per group: groups of `group`, last one partial
    sizes = [min(group, loops - g * group) for g in range((loops + group - 1) // group)]
    NGR = len(sizes)
    DPG = 3                  # dma_starts per group (diag / sub / super)

    nc = bass.Bass()
    l1d = nc.declare_dram_parameter("l1d", [P, group * LW1], fp8, isOutput=False)
    l0d = nc.declare_dram_parameter("l0d", [SUBP, group * LW1], fp8, isOutput=False)
    l2d = nc.declare_dram_parameter("l2d", [SUPP, group * LW1], fp8, isOutput=False)
    vblk = nc.declare_dram_parameter("vblk", [P, KB, MV], fp8, isOutput=False)
    # matmul operands must sit at partition base 0 (walrus: base in {0, 64}),
    # so the 96-row sub tiles pair with a +32-row-shifted copy of the V
    # table rather than a partition-offset slice of vblk.
    vshd = nc.declare_dram_parameter("vshd", [SUBP, KB, MV], fp8, isOutput=False)
    out = nc.declare_dram_parameter("partial", [P, 1], f32, isOutput=True)

    BGW = bufs * group * LW1

    with (
        nc.sbuf_tensor([P, KB, MV], fp8) as v_sb,
        nc.sbuf_tensor([SUBP, KB, MV], fp8) as vs_sb,
        nc.sbuf_tensor([P, BGW], fp8) as l1_sb,
        nc.sbuf_tensor([SUBP, BGW], fp8) as l0_sb,
        nc.sbuf_tensor([SUPP, BGW], fp8) as l2_sb,
        nc.psum_tensor([P, VPARTS, JB, M], f32) as acc,
        nc.sbuf_tensor([P, JB * M], f32) as tmp_sb,
        nc.sbuf_tensor([P, JB * M], f32) as comb_sb,
        nc.sbuf_tensor([P, JB * M], f32) as sq_sb,
        nc.sbuf_tensor([P, 1], f32) as red_sb,
        nc.semaphore("dma_sem") as dma_sem,
        nc.semaphore("pe_sem") as pe_sem,
        nc.semaphore("dve_sem") as dve_sem,
        nc.semaphore("out_sem") as out_sem,
    ):
        with nc.Block() as block:

            @block.sync
            def _(sync):
                sync.dma_start(v_sb[:], vblk[:]).then_inc(dma_sem, 16)
                sync.dma_start(vs_sb[:], vshd[:]).then_inc(dma_sem, 16)
                for g, sz in enumerate(sizes):
                    if g >= bufs:
                        sync.wait_ge(pe_sem, g - bufs + 1)
                    o = (g % bufs) * group * LW1
                    w = sz * LW1
                    sync.dma_start(
                        l1_sb[:, o : o + w], l1d[:, 0:w]
                    ).then_inc(dma_sem, 16)
                    sync.dma_start(
                        l0_sb[:, o : o + w], l0d[:, 0:w]
                    ).then_inc(dma_sem, 16)
                    sync.dma_start(
                        l2_sb[:, o : o + w], l2d[:, 0:w]
                    ).then_inc(dma_sem, 16)
                sync.wait_ge(dve_sem, 1)
                # Reset all waited-on semaphores BEFORE the out DMA: the
                # runtime can report execution done at out-buffer readiness,
                # so anything after the out DMA races the next execution of
                # the same loaded NEFF (sems are never cleared by the
                # runtime).  The out DMA gets its own never-waited sem.
                sync.wait_ge(dma_sem, 16 * (2 + DPG * NGR))
                for s in (dma_sem, pe_sem, dve_sem):
                    sync.sem_clear(s)
                sync.dma_start(out[:], red_sb[:]).then_inc(out_sem, 16)

            @block.tensor
            def _(tensor):
                for g, sz in enumerate(sizes):
                    tensor.wait_ge(dma_sem, 16 * (2 + DPG * (g + 1)))
                    mm = None
                    for gi in range(sz):
                        ro = ((g % bufs) * group + gi) * LW1
                        for j in range(JB):
                            c = ro + j * P
                            # kap=0: sub tile rows k=32..128 of block j-1,
                            # paired with the +32-shifted V table
                            mm = tensor.matmul(
                                acc[:, :, j, :],
                                l0_sb[:, c : c + P],
                                vs_sb[:, j, :],
                                start=(j == 0),
                                stop=False,
                            )
                            # kap=1: diagonal tile, full 128
                            mm = tensor.matmul(
                                acc[:, :, j, :],
                                l1_sb[:, c : c + P],
                                v_sb[:, j + 1, :],
                                start=False,
                                stop=False,
                            )
                            # kap=2: super tile rows k=0..96 of block j+1
                            mm = tensor.matmul(
                                acc[:, :, j, :],
                                l2_sb[:, c : c + P],
                                v_sb[0:SUPP, j + 2, :],
                                start=False,
                                stop=(j == JB - 1),
                            )
                    mm.then_inc(pe_sem, 1)

            @block.vector
            def _(vector):
                vector.wait_ge(pe_sem, NGR)
                vector.tensor_scalar_mul(
                    tmp_sb[:], acc[:, 1, :, :], 1.0 / TRI_SCALE
                )
                vector.tensor_add(comb_sb[:], acc[:, 0, :, :], tmp_sb[:])
                vector.tensor_mul(sq_sb[:], comb_sb[:], comb_sb[:])
                vector.reduce_sum(
                    red_sb[:], sq_sb[:], axis=mybir.AxisListType.X
                ).then_inc(dve_sem, 1)

    return nc


def _prepare_tri(laplacian, verts):
    import ml_dtypes

    fp8 = ml_dtypes.float8_e4m3
    L = np.asarray(laplacian, dtype=np.float32)
    V = np.asarray(verts, dtype=np.float32)
    assert L.shape == (N, N) and V.shape == (B, N, 3)

    Lb = L.astype(fp8).reshape(KTILES, P, KTILES, P)  # [gj, r, gk, c]
    V24 = V.transpose(1, 0, 2).reshape(N, M)
    hi = V24.astype(fp8)
    lo = ((V24 - hi.astype(np.float32)) * TRI_SCALE).astype(fp8)

    # k-block table with one zero guard block on each end (gk = -1, 72)
    vp = np.zeros((KTILES + 2, P, MV), fp8)
    vp[1 : KTILES + 1, :, :M] = hi.reshape(KTILES, P, M)
    vp[1 : KTILES + 1, :, M:] = lo.reshape(KTILES, P, M)

    SUB0 = TRI_SUB_BASE
    SUPP = TRI_SUP_SIZE
    in_maps = []
    for c in range(NCORES):
        l1 = np.zeros((P, JB, P), fp8)          # diag tiles   [k, j, m]
        l0 = np.zeros((P - SUB0, JB, P), fp8)   # sub tiles    [k-32, j, m]
        l2 = np.zeros((SUPP, JB, P), fp8)       # super tiles  [k, j, m]
        for j in range(JB):
            gj = JB * c + j
            l1[:, j, :] = Lb[gj, :, gj, :].T
            if gj - 1 >= 0:
                l0[:, j, :] = Lb[gj, :, gj - 1, :].T[SUB0:, :]
            if gj + 1 < KTILES:
                l2[:, j, :] = Lb[gj, :, gj + 1, :].T[:SUPP, :]
        vb = np.ascontiguousarray(vp[JB * c : JB * c + KB])  # [11, 128, 48]
        in_maps.append(
            {
                "l1d": np.tile(l1.reshape(P, JB * P), (1, TRI_GROUP)),
                "l0d": np.tile(l0.reshape(P - SUB0, JB * P), (1, TRI_GROUP)),
                "l2d": np.tile(l2.reshape(SUPP, JB * P), (1, TRI_GROUP)),
                "vblk": vb.transpose(1, 0, 2).copy(),           # [128, 11, 48]
                "vshd": vb[:, SUB0:, :].transpose(1, 0, 2).copy(),  # [96, 11, 48]
            }
        )
    return in_maps


def _is_block_tridiag(L):
    # The tri path (and its 96-partition off-diagonal tiles) requires every
    # nonzero of L within +-96 of the main diagonal.
    i, j = np.nonzero(L)
    if i.size == 0:
        return True
    return bool(np.abs(i - j).max() <= 96)


# --------------------------------------------------------------------------
# dense fallback: original full-matmul kernel
# --------------------------------------------------------------------------

def _build_nc(dtype_mode, loops=1):
    import concourse.bass as bass
    import concourse.mybir as mybir

    cfg = _MODES[dtype_mode]
    PARTS, GROUP, BUFS = cfg["parts"], cfg["group"], cfg["bufs"]
    MSTAT = PARTS * M
    NGROUPS = KTILES // GROUP
    dt_data = {
        "fp32": mybir.dt.float32,
        "bf16x2": mybir.dt.bfloat16,
        "fp8x4": mybir.dt.float8e4,
    }[dtype_mode]
    f32 = mybir.dt.float32
    GS = GROUP * SHARD
    split = PARTS > 1

    nc = bass.Bass()
    lcols = nc.declare_dram_parameter("lcols", [NGROUPS, P, GS], dt_data, isOutput=False)
    vstat = nc.declare_dram_parameter("vstat", [P, KTILES * MSTAT], dt_data, isOutput=False)
    out = nc.declare_dram_parameter("partial", [M, NCHUNK], f32, isOutput=True)

    with (
        nc.sbuf_tensor([P, KTILES * MSTAT], dt_data) as v_sb,
        nc.sbuf_tensor([P, BUFS * GS], dt_data) as l_sb,
        nc.psum_tensor([MSTAT, CHUNK], f32) as acc0,
        nc.psum_tensor([MSTAT, CHUNK], f32) as acc1,
        nc.psum_tensor([MSTAT, CHUNK], f32) as acc2,
        nc.sbuf_tensor([MSTAT, NCHUNK * CHUNK], f32) as cp_sb,
        nc.sbuf_tensor([M, max(PARTS - 1, 1) * NCHUNK * CHUNK], f32) as lo_sb,
        nc.sbuf_tensor([M, CHUNK], f32) as sq_sb,
        nc.sbuf_tensor([M, NCHUNK], f32) as red_sb,
        nc.semaphore("dma_sem") as dma_sem,
        nc.semaphore("pe_sem") as pe_sem,
        nc.semaphore("dvec_sem") as dvec_sem,
        nc.semaphore("dve_sem") as dve_sem,
        nc.semaphore("out_sem") as out_sem,
    ):
        accs = [acc0, acc1, acc2]
        NG = NGROUPS * loops
        MM_PER_G = GROUP * NCHUNK

        with nc.Block() as block:

            @block.sync
            def _(sync):
                sync.dma_start(v_sb[:], vstat[:]).then_inc(dma_sem, 16)
                for gu in range(NG):
                    u = gu % NGROUPS
                    if gu >= BUFS:
                        sync.wait_ge(pe_sem, MM_PER_G * (gu - BUFS + 1))
                    slot = gu % BUFS
                    sync.dma_start(
                        l_sb[:, slot * GS : (slot + 1) * GS], lcols[u]
                    ).then_inc(dma_sem, 16)
                if split:
                    # shift the lo accumulators down to partitions 0..23
                    sync.wait_ge(dvec_sem, NCHUNK)
                    for k in range(1, PARTS):
                        for j in range(NCHUNK):
                            o = ((k - 1) * NCHUNK + j) * CHUNK
                            sync.dma_start(
                                lo_sb[:, o : o + CHUNK],
                                cp_sb[k * M : (k + 1) * M, j * CHUNK : (j + 1) * CHUNK],
                            ).then_inc(dma_sem, 16)
                sync.wait_ge(dve_sem, 1)
                # Reset all waited-on semaphores BEFORE the out DMA: the
                # runtime can report execution done at out-buffer readiness,
                # so anything after the out DMA races the next execution of
                # the same loaded NEFF (sems are never cleared by the
                # runtime).  The out DMA gets its own never-waited sem.
                nshift = (PARTS - 1) * NCHUNK if split else 0
                sync.wait_ge(dma_sem, 16 * (1 + NG + nshift))
                for s in (dma_sem, pe_sem, dvec_sem, dve_sem):
                    sync.sem_clear(s)
                sync.dma_start(out[:], red_sb[:]).then_inc(out_sem, 16)

            @block.tensor
            def _(tensor):
                for gu in range(NG):
                    u = gu % NGROUPS
                    slot = gu % BUFS
                    tensor.wait_ge(dma_sem, 16 * (gu + 2))
                    for t_in in range(GROUP):
                        t = u * GROUP + t_in
                        for j in range(NCHUNK):
                            tensor.matmul(
                                accs[j][:],
                                v_sb[:, t * MSTAT : (t + 1) * MSTAT],
                                l_sb[
                                    :,
                                    slot * GS
                                    + t_in * SHARD
                                    + j * CHUNK : slot * GS
                                    + t_in * SHARD
                                    + (j + 1) * CHUNK,
                                ],
                                start=(t == 0),
                                stop=(t == KTILES - 1),
                            ).then_inc(pe_sem, 1)

            @block.vector
            def _(vector):
                vector.wait_ge(pe_sem, MM_PER_G * NG)
                if split:
                    for j in range(NCHUNK):
                        vector.tensor_copy(
                            cp_sb[:, j * CHUNK : (j + 1) * CHUNK], accs[j][:]
                        ).then_inc(dvec_sem, 1)
                    # lo parts arrive via the SP shift DMAs
                    nshift = (PARTS - 1) * NCHUNK
                    vector.wait_ge(dma_sem, 16 * (NG + 1 + nshift))
                    for j in range(NCHUNK):
                        acc = cp_sb[0:M, j * CHUNK : (j + 1) * CHUNK]
                        sc = SPLIT_SCALE if dtype_mode == "fp8x4" else 1.0
                        for k in range(1, PARTS):
                            o = ((k - 1) * NCHUNK + j) * CHUNK
                            lo = lo_sb[:, o : o + CHUNK]
                            if sc != 1.0:
                                vector.tensor_scalar_mul(lo, lo, 1.0 / sc**k)
                            vector.tensor_add(lo, acc, lo)
                            acc = lo
                        vector.tensor_mul(sq_sb[:], acc, acc)
                        red = vector.reduce_sum(
                            red_sb[:, j : j + 1], sq_sb[:], axis=mybir.AxisListType.X
                        )
                        if j == NCHUNK - 1:
                            red.then_inc(dve_sem, 1)
                else:
                    for j in range(NCHUNK):
                        cp = cp_sb[:, j * CHUNK : (j + 1) * CHUNK]
                        vector.tensor_copy(cp, accs[j][:])
                        vector.tensor_mul(sq_sb[:], cp, cp)
                        red = vector.reduce_sum(
                            red_sb[:, j : j + 1], sq_sb[:], axis=mybir.AxisListType.X
                        )
                        if j == NCHUNK - 1:
                            red.then_inc(dve_sem, 1)

    return nc


def _get_nc(dtype_mode, loops=1):
    key = (dtype_mode, loops)
    if key not in _cache:
        if dtype_mode == "tri":
            _cache[key] = _build_nc_tri(loops)
        else:
            _cache[key] = _build_nc(dtype_mode, loops)
    return _cache[key]


def _symmetric_sample(L, n=200000, seed=0):
    rng = np.random.default_rng(seed)
    i = rng.integers(0, L.shape[0], n)
    j = rng.integers(0, L.shape[1], n)
    return bool(np.array_equal(L[i, j], L[j, i]))


def _prepare_inputs(laplacian, verts, dtype_mode):
    import ml_dtypes

    if dtype_mode == "tri":
        return _prepare_tri(laplacian, verts)

    cfg = _MODES[dtype_mode]
    GROUP = cfg["group"]
    NGROUPS = KTILES // GROUP
    GS = GROUP * SHARD

    L = np.asarray(laplacian, dtype=np.float32)
    V = np.asarray(verts, dtype=np.float32)
    assert L.shape == (N, N) and V.shape == (B, N, 3)

    # rhs tiles need L^T columns; mesh Laplacians are symmetric so we can
    # slice L directly.  Sampled check with a transposed fallback keeps the
    # kernel correct for arbitrary (non-symmetric) inputs.
    Lsrc = L if _symmetric_sample(L) else np.ascontiguousarray(L.T)

    V24 = V.transpose(1, 0, 2).reshape(N, M)                    # [9216, 24]
    if dtype_mode == "fp32":
        vstat = np.ascontiguousarray(
            V24.reshape(KTILES, P, M).transpose(1, 0, 2)
        ).reshape(P, -1)
        Lcast = Lsrc
    else:
        dt = ml_dtypes.bfloat16 if dtype_mode == "bf16x2" else ml_dtypes.float8_e4m3
        sc = SPLIT_SCALE if dtype_mode == "fp8x4" else 1.0
        parts = _MODES[dtype_mode]["parts"]
        comps, resid = [], V24.copy()
        for k in range(parts):
            c = (resid * sc**k).astype(dt)
            comps.append(c.reshape(KTILES, P, M))
            resid = resid - c.astype(np.float32) / sc**k
        stat = np.concatenate(comps, axis=2)                     # [72,128,parts*24]
        vstat = np.ascontiguousarray(stat.transpose(1, 0, 2)).reshape(P, -1)
        Lcast = Lsrc.astype(dt)

    in_maps = []
    for c in range(NCORES):
        lc = np.ascontiguousarray(Lcast[:, c * SHARD : (c + 1) * SHARD])
        # interleave GROUP K-tiles side by side in the free dim
        lc = lc.reshape(NGROUPS, GROUP, P, SHARD).transpose(0, 2, 1, 3)
        lc = np.ascontiguousarray(lc).reshape(NGROUPS, P, GS)
        in_maps.append({"lcols": lc, "vstat": vstat})
    return in_maps


def _exact_in(L, dt):
    return bool(np.array_equal(L.astype(dt).astype(np.float32), L))


def kernel(laplacian, verts, _dtype_mode=None, _loops=1):
    import ml_dtypes
    from concourse.bass_utils import run_bass_kernel_spmd

    L = np.asarray(laplacian, dtype=np.float32)
    if _dtype_mode is None:
        # tri needs L block-tridiagonal and exactly fp8-representable
        # (mesh Laplacians: small-integer entries, bands within +-96).
        # The reduced-dtype dense kernels need only the exactness; fp32
        # is the always-correct fallback.
        if _exact_in(L, ml_dtypes.float8_e4m3):
            _dtype_mode = "tri" if _is_block_tridiag(L) else "fp8x4"
        elif _exact_in(L, ml_dtypes.bfloat16):
            _dtype_mode = "bf16x2"
        else:
            _dtype_mode = "fp32"

    in_maps = _prepare_inputs(L, verts, _dtype_mode)
    nc = _get_nc(_dtype_mode, _loops)
    res = run_bass_kernel_spmd(nc, in_maps, core_ids=list(range(NCORES)))
    total = np.float64(0.0)
    for r in res.results:
        total += r["partial"].astype(np.float64).sum()
    return np.float32(total / B)


# revision 15
# speedup vs baseline: 9.8580x; 1.7317x over previous
"""Trainium2 Bass kernel for MeshLaplacianLoss.

Computes  sum((L @ verts)**2) / B  for L [9216,9216] f32, verts [8,9216,3] f32.

Fast path "tri" (used when L is detected block-tridiagonal at 128
granularity and exactly representable in fp8e4m3 — true for mesh
Laplacians, whose 7 diagonals all sit within +-96 of the main): row-shard
the 9216 output rows over 8 cores (9 blocks of 128 rows each).  For
output block j only the three 128-wide column blocks k = j-1, j, j+1 of
L are nonzero, so core c streams just 27 transposed fp8 128x128 blocks
(432 KB vs 10.6 MB for the dense row-shard) and runs 27 accumulating
matmuls:

    lhsT (stationary) = L[j-rows, k-cols]^T  [128, 128] fp8
    rhs  (moving)     = V[k-block]           [128, 48]  fp8 (hi | lo*16)
    out  (PSUM)       = lv[j-block]          [128, 2, 9, 24] f32, one bank

V is split into fp8 hi + fp8 lo-scaled components (columns side by side)
to restore ~1e-5 relative accuracy; the whole per-core output lives in a
single PSUM bank, accumulated with one start / one stop (PSUM pending-
zero is bank granular).  Epilogue: combine hi + lo/16, square, free-dim
reduce to [128, 1] per core; host sums partials in float64 / B.

Dense fallback (any other L): the original full-matmul kernel, mode
fp8x4 / bf16x2 / fp32 chosen by exactness of L in the narrow dtype.

Written in raw Bass (explicit semaphores, hand-rolled multi-buffering):
the Tile scheduler's generated sync puts >1 semaphore wait on single
instructions, which this container's walrus rejects.
"""

import sys

for _p in ("/opt/trn_rl_repo",):
    if _p not in sys.path:
        sys.path.insert(0, _p)

import numpy as np

N = 9216
B = 8
NCORES = 8
SHARD = N // NCORES          # 1152 output rows per core
P = 128                      # partitions
KTILES = N // P              # 72
M = B * 3                    # 24 output columns of lv^T
NCHUNK = 3
CHUNK = SHARD // NCHUNK      # 384

# ---- tri (block-tridiagonal) path constants ----
JB = SHARD // P              # 9 output blocks per core
KB = JB + 2                  # 11 k-blocks incl halo
VPARTS = 2                   # fp8 hi + fp8 lo*16
MV = VPARTS * M              # 48 moving columns
TRI_BUFS = 2                 # L-block multibuffer depth (in groups)
TRI_GROUP = 8                # reps per dma_start / sem round-trip
TRI_SCALE = 16.0
# off-diagonal blocks only have bands within +-96 of the main diagonal:
# the kap=0 (sub) L^T blocks are zero in partitions < 32, the kap=2
# (super) blocks in partitions >= 96, so they ship as [96, 128] tiles.
TRI_SUB_BASE = 32            # kap=0 tiles live in partitions 32..128
TRI_SUP_SIZE = 96            # kap=2 tiles live in partitions 0..96

# parts: how many scaled components V is split into (cols = parts*24);
# GROUP K-tiles ride in one dma_start ([128, GROUP*1152]) to stay past the
# ~1 MiB DMA efficiency knee; BUFS slots give the prefetch depth.
# Component k of V is stored as dtype(v_k * SPLIT_SCALE**k) and the
# accumulators are recombined as sum_k out_k / SPLIT_SCALE**k.
SPLIT_SCALE = 16.0
_MODES = {
    "fp32": dict(parts=1, group=2, bufs=6),
    "bf16x2": dict(parts=2, group=4, bufs=6),
    "fp8x4": dict(parts=4, group=8, bufs=6),
}

_cache = {}


# --------------------------------------------------------------------------
# tri path: block-tridiagonal L, 27 fp8 128x128 blocks per core
# --------------------------------------------------------------------------

def _build_nc_tri(loops=1, group=TRI_GROUP, bufs=TRI_BUFS):
    import concourse.bass as bass
    import concourse.mybir as mybir

    fp8 = mybir.dt.float8e4
    f32 = mybir.dt.float32
    SUB0 = TRI_SUB_BASE      # 32
    SUBP = P - SUB0          # 96 partitions used by kap=0 tiles
    SUPP = TRI_SUP_SIZE      # 96 partitions used by kap=2 tiles
    LW1 = JB * P             # 1152 cols of diag tiles per rep
    # rep sizes per group: groups of `group`, last one partial
    sizes = [min(group, loops - g * group) for g in range((loops + group - 1) // group)]
    NGR = len(sizes)
    DPG = 3                  # dma_starts per group (diag / sub / super)

    nc = bass.Bass()
    l1d = nc.declare_dram_parameter("l1d", [P, group * LW1], fp8, isOutput=False)
    l0d = nc.declare_dram_parameter("l0d", [SUBP, group * LW1], fp8, isOutput=False)
    l2d = nc.declare_dram_parameter("l2d", [SUPP, group * LW1], fp8, isOutput=False)
    vblk = nc.declare_dram_parameter("vblk", [P, KB, MV], fp8, isOutput=False)
    # matmul operands must sit at partition base 0 (walrus: base in {0, 64}),
    # so the 96-row sub tiles pair with a +32-row-shifted copy of the V
    # table rather than a partition-offset slice of vblk.
    vshd = nc.declare_dram_parameter("vshd", [SUBP, KB, MV], fp8, isOutput=False)
    out = nc.declare_dram_parameter("partial", [P, 1], f32, isOutput=True)

    BGW = bufs * group * LW1

    with (
        nc.sbuf_tensor([P, KB, MV], fp8) as v_sb,
        nc.sbuf_tensor([SUBP, KB, MV], fp8) as vs_sb,
        nc.sbuf_tensor([P, BGW], fp8) as l1_sb,
        nc.sbuf_tensor([SUBP, BGW], fp8) as l0_sb,
        nc.sbuf_tensor([SUPP, BGW], fp8) as l2_sb,
        nc.psum_tensor([P, VPARTS, JB, M], f32) as acc,
        nc.sbuf_tensor([P, JB * M], f32) as tmp_sb,
        nc.sbuf_tensor([P, JB * M], f32) as comb_sb,
        nc.sbuf_tensor([P, JB * M], f32) as sq_sb,
        nc.sbuf_tensor([P, 1], f32) as red_sb,
        nc.semaphore("dma_sem") as dma_sem,
        nc.semaphore("pe_sem") as pe_sem,
        nc.semaphore("dve_sem") as dve_sem,
        nc.semaphore("out_sem") as out_sem,
    ):
        with nc.Block() as block:

            @block.sync
            def _(sync):
                sync.dma_start(v_sb[:], vblk[:]).then_inc(dma_sem, 16)
                sync.dma_start(vs_sb[:], vshd[:]).then_inc(dma_sem, 16)
                for g, sz in enumerate(sizes):
                    if g >= bufs:
                        sync.wait_ge(pe_sem, g - bufs + 1)
                    o = (g % bufs) * group * LW1
                    w = sz * LW1
                    sync.dma_start(
                        l1_sb[:, o : o + w], l1d[:, 0:w]
                    ).then_inc(dma_sem, 16)
                    sync.dma_start(
                        l0_sb[:, o : o + w], l0d[:, 0:w]
                    ).then_inc(dma_sem, 16)
                    sync.dma_start(
                        l2_sb[:, o : o + w], l2d[:, 0:w]
                    ).then_inc(dma_sem, 16)
                sync.wait_ge(dve_sem, 1)
                # Reset all waited-on semaphores BEFORE the out DMA: the
                # runtime can report execution done at out-buffer readiness,
                # so anything after the out DMA races the next execution of
                # the same loaded NEFF (sems are never cleared by the
                # runtime).  The out DMA gets its own never-waited sem.
                sync.wait_ge(dma_sem, 16 * (2 + DPG * NGR))
                for s in (dma_sem, pe_sem, dve_sem):
                    sync.sem_clear(s)
                sync.dma_start(out[:], red_sb[:]).then_inc(out_sem, 16)

            @block.tensor
            def _(tensor):
                for g, sz in enumerate(sizes):
                    tensor.wait_ge(dma_sem, 16 * (2 + DPG * (g + 1)))
                    mm = None
                    for gi in range(sz):
                        ro = ((g % bufs) * group + gi) * LW1
                        for j in range(JB):
                            c = ro + j * P
                            # kap=0: sub tile rows k=32..128 of block j-1,
                            # paired with the +32-shifted V table
                            mm = tensor.matmul(
                                acc[:, :, j, :],
                                l0_sb[:, c : c + P],
                                vs_sb[:, j, :],
                                start=(j == 0),
                                stop=False,
                            )
                            # kap=1: diagonal tile, full 128
                            mm = tensor.matmul(
                                acc[:, :, j, :],
                                l1_sb[:, c : c + P],
                                v_sb[:, j + 1, :],
                                start=False,
                                stop=False,
                            )
                            # kap=2: super tile rows k=0..96 of block j+1
                            mm = tensor.matmul(
                                acc[:, :, j, :],
                                l2_sb[:, c : c + P],
                                v_sb[0:SUPP, j + 2, :],
                                start=False,
                                stop=(j == JB - 1),
                            )
                    mm.then_inc(pe_sem, 1)

            @block.vector
            def _(vector):
                vector.wait_ge(pe_sem, NGR)
                vector.tensor_scalar_mul(
                    tmp_sb[:], acc[:, 1, :, :], 1.0 / TRI_SCALE
                )
                vector.tensor_add(comb_sb[:], acc[:, 0, :, :], tmp_sb[:])
                vector.tensor_mul(sq_sb[:], comb_sb[:], comb_sb[:])
                vector.reduce_sum(
                    red_sb[:], sq_sb[:], axis=mybir.AxisListType.X
                ).then_inc(dve_sem, 1)

    return nc


def _prepare_tri(laplacian, verts, group=TRI_GROUP):
    import ml_dtypes

    fp8 = ml_dtypes.float8_e4m3
    L = np.asarray(laplacian, dtype=np.float32)
    V = np.asarray(verts, dtype=np.float32)
    assert L.shape == (N, N) and V.shape == (B, N, 3)

    Lb = L.astype(fp8).reshape(KTILES, P, KTILES, P)  # [gj, r, gk, c]
    V24 = V.transpose(1, 0, 2).reshape(N, M)
    hi = V24.astype(fp8)
    lo = ((V24 - hi.astype(np.float32)) * TRI_SCALE).astype(fp8)

    # k-block table with one zero guard block on each end (gk = -1, 72)
    vp = np.zeros((KTILES + 2, P, MV), fp8)
    vp[1 : KTILES + 1, :, :M] = hi.reshape(KTILES, P, M)
    vp[1 : KTILES + 1, :, M:] = lo.reshape(KTILES, P, M)

    SUB0 = TRI_SUB_BASE
    SUPP = TRI_SUP_SIZE
    in_maps = []
    for c in range(NCORES):
        l1 = np.zeros((P, JB, P), fp8)          # diag tiles   [k, j, m]
        l0 = np.zeros((P - SUB0, JB, P), fp8)   # sub tiles    [k-32, j, m]
        l2 = np.zeros((SUPP, JB, P), fp8)       # super tiles  [k, j, m]
        for j in range(JB):
            gj = JB * c + j
            l1[:, j, :] = Lb[gj, :, gj, :].T
            if gj - 1 >= 0:
                l0[:, j, :] = Lb[gj, :, gj - 1, :].T[SUB0:, :]
            if gj + 1 < KTILES:
                l2[:, j, :] = Lb[gj, :, gj + 1, :].T[:SUPP, :]
        vb = np.ascontiguousarray(vp[JB * c : JB * c + KB])  # [11, 128, 48]
        in_maps.append(
            {
                "l1d": np.tile(l1.reshape(P, JB * P), (1, group)),
                "l0d": np.tile(l0.reshape(P - SUB0, JB * P), (1, group)),
                "l2d": np.tile(l2.reshape(SUPP, JB * P), (1, group)),
                "vblk": vb.transpose(1, 0, 2).copy(),           # [128, 11, 48]
                "vshd": vb[:, SUB0:, :].transpose(1, 0, 2).copy(),  # [96, 11, 48]
            }
        )
    return in_maps


def _is_block_tridiag(L):
    # The tri path (and its 96-partition off-diagonal tiles) requires every
    # nonzero of L within +-96 of the main diagonal.
    i, j = np.nonzero(L)
    if i.size == 0:
        return True
    return bool(np.abs(i - j).max() <= 96)


# --------------------------------------------------------------------------
# dense fallback: original full-matmul kernel
# --------------------------------------------------------------------------

def _build_nc(dtype_mode, loops=1):
    import concourse.bass as bass
    import concourse.mybir as mybir

    cfg = _MODES[dtype_mode]
    PARTS, GROUP, BUFS = cfg["parts"], cfg["group"], cfg["bufs"]
    MSTAT = PARTS * M
    NGROUPS = KTILES // GROUP
    dt_data = {
        "fp32": mybir.dt.float32,
        "bf16x2": mybir.dt.bfloat16,
        "fp8x4": mybir.dt.float8e4,
    }[dtype_mode]
    f32 = mybir.dt.float32
    GS = GROUP * SHARD
    split = PARTS > 1

    nc = bass.Bass()
    lcols = nc.declare_dram_parameter("lcols", [NGROUPS, P, GS], dt_data, isOutput=False)
    vstat = nc.declare_dram_parameter("vstat", [P, KTILES * MSTAT], dt_data, isOutput=False)
    out = nc.declare_dram_parameter("partial", [M, NCHUNK], f32, isOutput=True)

    with (
        nc.sbuf_tensor([P, KTILES * MSTAT], dt_data) as v_sb,
        nc.sbuf_tensor([P, BUFS * GS], dt_data) as l_sb,
        nc.psum_tensor([MSTAT, CHUNK], f32) as acc0,
        nc.psum_tensor([MSTAT, CHUNK], f32) as acc1,
        nc.psum_tensor([MSTAT, CHUNK], f32) as acc2,
        nc.sbuf_tensor([MSTAT, NCHUNK * CHUNK], f32) as cp_sb,
        nc.sbuf_tensor([M, max(PARTS - 1, 1) * NCHUNK * CHUNK], f32) as lo_sb,
        nc.sbuf_tensor([M, CHUNK], f32) as sq_sb,
        nc.sbuf_tensor([M, NCHUNK], f32) as red_sb,
        nc.semaphore("dma_sem") as dma_sem,
        nc.semaphore("pe_sem") as pe_sem,
        nc.semaphore("dvec_sem") as dvec_sem,
        nc.semaphore("dve_sem") as dve_sem,
        nc.semaphore("out_sem") as out_sem,
    ):
        accs = [acc0, acc1, acc2]
        NG = NGROUPS * loops
        MM_PER_G = GROUP * NCHUNK

        with nc.Block() as block:

            @block.sync
            def _(sync):
                sync.dma_start(v_sb[:], vstat[:]).then_inc(dma_sem, 16)
                for gu in range(NG):
                    u = gu % NGROUPS
                    if gu >= BUFS:
                        sync.wait_ge(pe_sem, MM_PER_G * (gu - BUFS + 1))
                    slot = gu % BUFS
                    sync.dma_start(
                        l_sb[:, slot * GS : (slot + 1) * GS], lcols[u]
                    ).then_inc(dma_sem, 16)
                if split:
                    # shift the lo accumulators down to partitions 0..23
                    sync.wait_ge(dvec_sem, NCHUNK)
                    for k in range(1, PARTS):
                        for j in range(NCHUNK):
                            o = ((k - 1) * NCHUNK + j) * CHUNK
                            sync.dma_start(
                                lo_sb[:, o : o + CHUNK],
                                cp_sb[k * M : (k + 1) * M, j * CHUNK : (j + 1) * CHUNK],
                            ).then_inc(dma_sem, 16)
                sync.wait_ge(dve_sem, 1)
                # Reset all waited-on semaphores BEFORE the out DMA: the
                # runtime can report execution done at out-buffer readiness,
                # so anything after the out DMA races the next execution of
                # the same loaded NEFF (sems are never cleared by the
                # runtime).  The out DMA gets its own never-waited sem.
                nshift = (PARTS - 1) * NCHUNK if split else 0
                sync.wait_ge(dma_sem, 16 * (1 + NG + nshift))
                for s in (dma_sem, pe_sem, dvec_sem, dve_sem):
                    sync.sem_clear(s)
                sync.dma_start(out[:], red_sb[:]).then_inc(out_sem, 16)

            @block.tensor
            def _(tensor):
                for gu in range(NG):
                    u = gu % NGROUPS
                    slot = gu % BUFS
                    tensor.wait_ge(dma_sem, 16 * (gu + 2))
                    for t_in in range(GROUP):
                        t = u * GROUP + t_in
                        for j in range(NCHUNK):
                            tensor.matmul(
                                accs[j][:],
                                v_sb[:, t * MSTAT : (t + 1) * MSTAT],
                                l_sb[
                                    :,
                                    slot * GS
                                    + t_in * SHARD
                                    + j * CHUNK : slot * GS
                                    + t_in * SHARD
                                    + (j + 1) * CHUNK,
                                ],
                                start=(t == 0),
                                stop=(t == KTILES - 1),
                            ).then_inc(pe_sem, 1)

            @block.vector
            def _(vector):
                vector.wait_ge(pe_sem, MM_PER_G * NG)
                if split:
                    for j in range(NCHUNK):
                        vector.tensor_copy(
                            cp_sb[:, j * CHUNK : (j + 1) * CHUNK], accs[j][:]
                        ).then_inc(dvec_sem, 1)
                    # lo parts arrive via the SP shift DMAs
                    nshift = (PARTS - 1) * NCHUNK
                    vector.wait_ge(dma_sem, 16 * (NG + 1 + nshift))
                    for j in range(NCHUNK):
                        acc = cp_sb[0:M, j * CHUNK : (j + 1) * CHUNK]
                        sc = SPLIT_SCALE if dtype_mode == "fp8x4" else 1.0
                        for k in range(1, PARTS):
                            o = ((k - 1) * NCHUNK + j) * CHUNK
                            lo = lo_sb[:, o : o + CHUNK]
                            if sc != 1.0:
                                vector.tensor_scalar_mul(lo, lo, 1.0 / sc**k)
                            vector.tensor_add(lo, acc, lo)
                            acc = lo
                        vector.tensor_mul(sq_sb[:], acc, acc)
                        red = vector.reduce_sum(
                            red_sb[:, j : j + 1], sq_sb[:], axis=mybir.AxisListType.X
                        )
                        if j == NCHUNK - 1:
                            red.then_inc(dve_sem, 1)
                else:
                    for j in range(NCHUNK):
                        cp = cp_sb[:, j * CHUNK : (j + 1) * CHUNK]
                        vector.tensor_copy(cp, accs[j][:])
                        vector.tensor_mul(sq_sb[:], cp, cp)
                        red = vector.reduce_sum(
                            red_sb[:, j : j + 1], sq_sb[:], axis=mybir.AxisListType.X
                        )
                        if j == NCHUNK - 1:
                            red.then_inc(dve_sem, 1)

    return nc


def _get_nc(dtype_mode, loops=1):
    key = (dtype_mode, loops)
    if key not in _cache:
        if dtype_mode == "tri":
            _cache[key] = _build_nc_tri(loops)
        else:
            _cache[key] = _build_nc(dtype_mode, loops)
    return _cache[key]


def _symmetric_sample(L, n=200000, seed=0):
    rng = np.random.default_rng(seed)
    i = rng.integers(0, L.shape[0], n)
    j = rng.integers(0, L.shape[1], n)
    return bool(np.array_equal(L[i, j], L[j, i]))


def _prepare_inputs(laplacian, verts, dtype_mode):
    import ml_dtypes

    if dtype_mode == "tri":
        return _prepare_tri(laplacian, verts)

    cfg = _MODES[dtype_mode]
    GROUP = cfg["group"]
    NGROUPS = KTILES // GROUP
    GS = GROUP * SHARD

    L = np.asarray(laplacian, dtype=np.float32)
    V = np.asarray(verts, dtype=np.float32)
    assert L.shape == (N, N) and V.shape == (B, N, 3)

    # rhs tiles need L^T columns; mesh Laplacians are symmetric so we can
    # slice L directly.  Sampled check with a transposed fallback keeps the
    # kernel correct for arbitrary (non-symmetric) inputs.
    Lsrc = L if _symmetric_sample(L) else np.ascontiguousarray(L.T)

    V24 = V.transpose(1, 0, 2).reshape(N, M)                    # [9216, 24]
    if dtype_mode == "fp32":
        vstat = np.ascontiguousarray(
            V24.reshape(KTILES, P, M).transpose(1, 0, 2)
        ).reshape(P, -1)
        Lcast = Lsrc
    else:
        dt = ml_dtypes.bfloat16 if dtype_mode == "bf16x2" else ml_dtypes.float8_e4m3
        sc = SPLIT_SCALE if dtype_mode == "fp8x4" else 1.0
        parts = _MODES[dtype_mode]["parts"]
        comps, resid = [], V24.copy()
        for k in range(parts):
            c = (resid * sc**k).astype(dt)
            comps.append(c.reshape(KTILES, P, M))
            resid = resid - c.astype(np.float32) / sc**k
        stat = np.concatenate(comps, axis=2)                     # [72,128,parts*24]
        vstat = np.ascontiguousarray(stat.transpose(1, 0, 2)).reshape(P, -1)
        Lcast = Lsrc.astype(dt)

    in_maps = []
    for c in range(NCORES):
        lc = np.ascontiguousarray(Lcast[:, c * SHARD : (c + 1) * SHARD])
        # interleave GROUP K-tiles side by side in the free dim
        lc = lc.reshape(NGROUPS, GROUP, P, SHARD).transpose(0, 2, 1, 3)
        lc = np.ascontiguousarray(lc).reshape(NGROUPS, P, GS)
        in_maps.append({"lcols": lc, "vstat": vstat})
    return in_maps


def _exact_in(L, dt):
    return bool(np.array_equal(L.astype(dt).astype(np.float32), L))


def kernel(laplacian, verts, _dtype_mode=None, _loops=1):
    import ml_dtypes
    from concourse.bass_utils import run_bass_kernel_spmd

    L = np.asarray(laplacian, dtype=np.float32)
    if _dtype_mode is None:
        # tri needs L block-tridiagonal and exactly fp8-representable
        # (mesh Laplacians: small-integer entries, bands within +-96).
        # The reduced-dtype dense kernels need only the exactness; fp32
        # is the always-correct fallback.
        if _exact_in(L, ml_dtypes.float8_e4m3):
            _dtype_mode = "tri" if _is_block_tridiag(L) else "fp8x4"
        elif _exact_in(L, ml_dtypes.bfloat16):
            _dtype_mode = "bf16x2"
        else:
            _dtype_mode = "fp32"

    in_maps = _prepare_inputs(L, verts, _dtype_mode)
    nc = _get_nc(_dtype_mode, _loops)
    res = run_bass_kernel_spmd(nc, in_maps, core_ids=list(range(NCORES)))
    total = np.float64(0.0)
    for r in res.results:
        total += r["partial"].astype(np.float64).sum()
    return np.float32(total / B)
